# revision 17
# baseline (speedup 1.0000x reference)
"""MoE MLP kernel for 8 Trainium2 NeuronCores.

Strategy (expert-parallel, per the sharding hint):
  - Router (tiny: [8192,1024]@[1024,8]) + top-2 selection + aux losses run on
    host in numpy — this is the dispatch step.
  - Token dispatch ("all-to-all") happens host-side while sharding: for each
    expert e, the tokens routed to it are gathered into a padded x^T block.
  - Core e runs the dense expert MLP  y_e^T = w2_e^T @ relu(w1_e^T @ x_e^T)^2
    entirely on device (float32r matmuls at full PE rate).
  - Unshard: host scatter-adds  y[t] += combine[t,e] * y_e[t]  over the top-2
    experts of each token.

Only top-2 of 8 experts are computed per token (the reference computes all 8
densely then weights unselected experts by 0), cutting device FLOPs 4x.
"""

from contextlib import ExitStack

import numpy as np

import concourse.bass as bass
import concourse.mybir as mybir
import concourse.tile as tile
from concourse.bass_utils import run_bass_kernel_spmd

B, S, D, E, H, K = 4, 2048, 1024, 8, 1024, 2
T = B * S
NCORES = 8
P = 128  # partitions
DC = D // P  # 8 d-chunks
HC = H // P  # 8 h-chunks

F32 = mybir.dt.float32
MM_DT = mybir.dt.float32r  # full-rate fp32 matmul mode (N>=256)

_program_cache = {}


def _build_program(C, tiles):
    """One expert's dense MLP: yT = w2^T @ relu(w1^T @ xT)^2, xT/yT [D, C].

    `tiles` is the list of token-tile widths (each 256 or 512, summing to C).
    """
    nc = bass.Bass()
    xg = nc.declare_dram_parameter("xg", [D, C], MM_DT, isOutput=False)
    w1 = nc.declare_dram_parameter("w1", [D, H], MM_DT, isOutput=False)
    w2 = nc.declare_dram_parameter("w2", [H, D], MM_DT, isOutput=False)
    yTs = [
        nc.declare_dram_parameter(f"yT_{tt}", [D, n], F32, isOutput=True)
        for tt, n in enumerate(tiles)
    ]

    # DRAM views with the 128-partition chunk structure exposed:
    # [p, chunk, col] where row = chunk*128 + p
    xg_v = xg[:].rearrange("(a p) c -> p a c", p=P)
    w1_v = w1[:].rearrange("(a p) h -> p a h", p=P)
    w2_v = w2[:].rearrange("(a p) d -> p a d", p=P)
    yT_vs = [t[:].rearrange("(a p) c -> p a c", p=P) for t in yTs]

    with tile.TileContext(nc) as tc:
        with ExitStack() as ctx:
            wpool = ctx.enter_context(tc.tile_pool(name="weights", bufs=1))
            xpool = ctx.enter_context(tc.tile_pool(name="xin", bufs=1))
            hpool = ctx.enter_context(tc.tile_pool(name="hid", bufs=2))
            ypool = ctx.enter_context(tc.tile_pool(name="yout", bufs=1))
            ph_pool = ctx.enter_context(
                tc.tile_pool(name="psum_h", bufs=2, space="PSUM")
            )
            py_pool = ctx.enter_context(
                tc.tile_pool(name="psum_y", bufs=4, space="PSUM")
            )
            pp_pool = ctx.enter_context(
                tc.tile_pool(name="psum_probe", bufs=1, space="PSUM")
            )

            # The matmul codegen allows only ONE sync-wait command. A "probe"
            # is a tiny PE matmul issued right after an input DMA: it carries
            # that DMA-queue wait on PE's vector clock, so the real matmuls
            # reading the data need no DMA wait of their own.
            probe_ps = pp_pool.tile([P, 8], F32, tag="pp", name="probe_ps")
            probe_n = [0]

            def probe(ap2d):
                nc.tensor.matmul(
                    probe_ps[:1, :8],
                    ap2d[:, :1],
                    ap2d[:, :8],
                    start=True,
                    stop=True,
                )
                probe_n[0] += 1

            # Resident weights: 8 tiles of [128, 1024] each for w1 and w2.
            # x^T is fully resident too (no slot reuse -> minimal sync waits);
            # its DMA is split per token-tile so early matmuls start sooner.
            w1_sb = []
            w2_sb = []
            for c in range(DC):
                t1 = wpool.tile([P, H], MM_DT, tag=f"w1_{c}", name=f"w1_{c}")
                nc.sync.dma_start(out=t1, in_=w1_v[:, c, :])
                probe(t1)
                w1_sb.append(t1)
            x_sb = xpool.tile([P, DC, C], MM_DT, tag="x", name="x_all")
            col = 0
            for tt, N in enumerate(tiles):
                nc.sync.dma_start(
                    out=x_sb[:, :, col : col + N], in_=xg_v[:, :, col : col + N]
                )
                probe(x_sb[:, 0, col : col + N])
                if tt == 0:
                    for c in range(HC):
                        t2 = wpool.tile([P, D], MM_DT, tag=f"w2_{c}", name=f"w2_{c}")
                        nc.sync.dma_start(out=t2, in_=w2_v[:, c, :])
                        probe(t2)
                        w2_sb.append(t2)
                col += N

            col = 0
            for tt, N in enumerate(tiles):
                csl = slice(col, col + N)
                col += N

                # First layer + relu^2 (in place) -> h_sb [128, 8(hc), N]
                h_sb = hpool.tile([P, HC, N], MM_DT, tag="h", name=f"h_{tt}")
                for hc in range(HC):
                    ph = ph_pool.tile([P, N], F32, tag="ph", name=f"ph_{tt}_{hc}")
                    for dc in range(DC):
                        nc.tensor.matmul(
                            ph,
                            w1_sb[dc][:, bass.ts(hc, P)],
                            x_sb[:, dc, csl],
                            start=(dc == 0),
                            stop=(dc == DC - 1),
                        )
                    hs = h_sb[:, hc, :]
                    nc.vector.tensor_scalar_max(hs, ph, 0.0)
                    nc.vector.tensor_mul(hs, hs, hs)

                # Second layer: one d-chunk (one PSUM bank) per group;
                # copies gather into one SBUF tile, one DMA per token-tile
                # into its own output tensor (no cross-queue WAW waits).
                y_sb = ypool.tile([P, DC, N], F32, tag="y", name=f"y_{tt}")
                # DVE pre-op: absorbs the WAR wait on y_sb (previous tile's
                # outbound DMA) so each real copy carries only its PE wait.
                nc.vector.tensor_copy(y_sb[:1, 0, :1], y_sb[:1, 0, :1])
                for dc in range(DC):
                    py = py_pool.tile([P, N], F32, tag="py", name=f"py_{tt}_{dc}")
                    for hc in range(HC):
                        nc.tensor.matmul(
                            py,
                            w2_sb[hc][:, bass.ts(dc, P)],
                            h_sb[:, hc, :],
                            start=(hc == 0),
                            stop=(hc == HC - 1),
                        )
                    nc.vector.tensor_copy(y_sb[:, dc, :], py)
                nc.gpsimd.dma_start(out=yT_vs[tt], in_=y_sb)

    _split_multi_waits(nc)
    return nc


def _split_multi_waits(nc):
    """This walrus build allows only ONE sync-wait command per instruction.
    Hoist all but the last wait of any multi-wait instruction onto fresh
    same-engine NoOps inserted immediately before it (sequencer executes
    them in order, so semantics are identical)."""
    n = [0]
    for f in nc.m.functions:
        for blk in f.blocks:
            out = []
            for inst in blk.instructions:
                si = inst.sync_info
                if si is not None and len(si.on_wait) > 1:
                    waits = list(si.on_wait)
                    for w in waits[:-1]:
                        nop = mybir.InstNoOp(
                            name=f"I-waitsplit-{n[0]}", ins=[], outs=[]
                        )
                        n[0] += 1
                        nop.engine = inst.engine
                        nop.sync_info = mybir.SyncInfo(
                            on_wait=[w], on_update=[]
                        )
                        out.append(nop)
                    inst.sync_info = mybir.SyncInfo(
                        on_wait=[waits[-1]], on_update=list(si.on_update)
                    )
                out.append(inst)
            blk.instructions = out


def _route(x_flat, router_w, expert_bias):
    """Replicates the reference router in numpy float32."""
    logits = x_flat @ router_w.T  # [T, E]
    probs = 1.0 / (1.0 + np.exp(-logits, dtype=np.float32))
    scores = probs + expert_bias[None, :]
    # jax.lax.top_k: descending order, ties -> lower index first
    sel = np.argsort(-scores, axis=-1, kind="stable")[:, :K]  # [T, K]
    topw = np.take_along_axis(probs, sel, axis=-1)
    topw = topw / (topw.sum(-1, keepdims=True) + 1e-20)
    return logits, probs, sel, topw


def kernel(x, router_w, w1, w2, expert_bias):
    x = np.asarray(x, dtype=np.float32)
    router_w = np.asarray(router_w, dtype=np.float32)
    w1 = np.asarray(w1, dtype=np.float32)
    w2 = np.asarray(w2, dtype=np.float32)
    expert_bias = np.asarray(expert_bias, dtype=np.float32)

    b, s, d = x.shape
    x_flat = x.reshape(-1, d)
    n_tok = x_flat.shape[0]

    logits, probs, sel, topw = _route(x_flat, router_w, expert_bias)

    # Token dispatch: gather per-expert token lists
    idx = [np.nonzero((sel == e).any(axis=-1))[0] for e in range(E)]
    counts = [len(i) for i in idx]
    max_count = max(max(counts), 1)

    # Capacity: round up to a multiple of 256; tile widths 512 with an
    # optional trailing 256 (N>=256 keeps float32r at full PE rate).
    C = -(-max_count // 256) * 256
    if C % 512 == 256 and C > 256:
        tiles = [512] * (C // 512) + [256]
    elif C == 256:
        C = 512
        tiles = [512]
    else:
        tiles = [512] * (C // 512)

    key = (C, tuple(tiles))
    if key not in _program_cache:
        _program_cache[key] = _build_program(C, tiles)
    nc = _program_cache[key]

    xT = np.ascontiguousarray(x_flat.T)  # [D, T]
    in_maps = []
    for e in range(E):
        xg = np.zeros((D, C), dtype=np.float32)
        xg[:, : counts[e]] = xT[:, idx[e]]
        in_maps.append(
            {
                "xg": xg,
                "w1": np.ascontiguousarray(w1[:, e * H : (e + 1) * H]),
                "w2": np.ascontiguousarray(w2[e * H : (e + 1) * H, :]),
            }
        )

    res = run_bass_kernel_spmd(nc, in_maps, list(range(NCORES)))
    global LAST_RESULTS
    LAST_RESULTS = res

    # Unshard: scatter-add with combine weights
    out_flat = np.zeros((n_tok, d), dtype=np.float32)
    for e in range(E):
        if counts[e] == 0:
            continue
        yT_e = np.concatenate(
            [res.results[e][f"yT_{tt}"] for tt in range(len(tiles))], axis=1
        )  # [D, C]
        c_e = np.where(sel[idx[e]] == e, topw[idx[e]], 0.0).sum(-1)
        out_flat[idx[e]] += c_e[:, None].astype(np.float32) * yT_e[:, : counts[e]].T

    output = out_flat.reshape(b, s, d)

    # Aux losses (host, matching the reference formulas)
    m = logits.max(-1)
    lse = m + np.log(np.exp(logits - m[:, None]).sum(-1))
    router_z_loss = np.float32(np.mean(np.square(lse)))

    sel_flat = sel.reshape(-1)
    tokens_per_expert = np.zeros(E, np.float32)
    np.add.at(tokens_per_expert, sel_flat, 1.0)
    f_i = (tokens_per_expert / tokens_per_expert.sum()).astype(np.float32)

    sel_seq = sel.reshape(b, s * K)
    f_seq = np.zeros((b, E), np.float32)
    for i in range(b):
        np.add.at(f_seq[i], sel_seq[i], 1.0)
    f_seq /= s * K
    p_seq = probs.reshape(b, s, E).mean(axis=1)
    load_balance_loss = np.float32(E * (f_seq * p_seq).sum(-1).mean())

    compute_loss = np.float32(probs.sum(-1).mean())

    return output, router_z_loss, load_balance_loss, compute_loss, f_i


# revision 18
# speedup vs baseline: 1.0029x; 1.0029x over previous
"""MoE MLP kernel for 8 Trainium2 NeuronCores.

Strategy (expert-parallel, per the sharding hint):
  - Router (tiny: [8192,1024]@[1024,8]) + top-2 selection + aux losses run on
    host in numpy — this is the dispatch step.
  - Token dispatch ("all-to-all") happens host-side while sharding: for each
    expert e, the tokens routed to it are gathered into a padded x^T block.
  - Core e runs the dense expert MLP  y_e^T = w2_e^T @ relu(w1_e^T @ x_e^T)^2
    entirely on device (float32r matmuls at full PE rate).
  - Unshard: host scatter-adds  y[t] += combine[t,e] * y_e[t]  over the top-2
    experts of each token.

Only top-2 of 8 experts are computed per token (the reference computes all 8
densely then weights unselected experts by 0), cutting device FLOPs 4x.
"""

from contextlib import ExitStack

import numpy as np

import concourse.bass as bass
import concourse.mybir as mybir
import concourse.tile as tile
from concourse.bass_utils import run_bass_kernel_spmd

# If BASS_TRACE is set, bass_utils imports antenv.axon_hooks, which is absent
# from this image. Install a shim so tracing degrades gracefully (and works,
# when trn_agent_boot can drive NTFF profiling via ctypes).
try:
    import antenv.axon_hooks  # noqa: F401
except ImportError:
    try:
        import sys as _sys
        import types as _types

        import antenv as _antenv

        _mod = _types.ModuleType("antenv.axon_hooks")
        _mod._hook = None
        _mod.set_axon_ntff_profile_hook = lambda h: setattr(_mod, "_hook", h)
        _mod.get_axon_ntff_profile_hook = lambda: _mod._hook
        _sys.modules["antenv.axon_hooks"] = _mod
        _antenv.axon_hooks = _mod
        try:
            from trn_agent_boot.trn_boot import _ntff_profile_via_ctypes

            _mod.set_axon_ntff_profile_hook(
                _ntff_profile_via_ctypes("/opt/axon/libaxon_pjrt.so")
            )
        except Exception:
            pass
    except Exception:
        pass

B, S, D, E, H, K = 4, 2048, 1024, 8, 1024, 2
T = B * S
NCORES = 8
P = 128  # partitions
DC = D // P  # 8 d-chunks
HC = H // P  # 8 h-chunks

F32 = mybir.dt.float32
MM_DT = mybir.dt.float32r  # full-rate fp32 matmul mode (N>=256)

_program_cache = {}


def _build_program(C, tiles):
    """One expert's dense MLP: yT = w2^T @ relu(w1^T @ xT)^2, xT/yT [D, C].

    `tiles` is the list of token-tile widths (each 256 or 512, summing to C).
    """
    nc = bass.Bass()
    xg = nc.declare_dram_parameter("xg", [D, C], MM_DT, isOutput=False)
    w1 = nc.declare_dram_parameter("w1", [D, H], MM_DT, isOutput=False)
    w2 = nc.declare_dram_parameter("w2", [H, D], MM_DT, isOutput=False)
    yTs = [
        nc.declare_dram_parameter(f"yT_{tt}", [D, n], F32, isOutput=True)
        for tt, n in enumerate(tiles)
    ]

    # DRAM views with the 128-partition chunk structure exposed:
    # [p, chunk, col] where row = chunk*128 + p
    xg_v = xg[:].rearrange("(a p) c -> p a c", p=P)
    w1_v = w1[:].rearrange("(a p) h -> p a h", p=P)
    w2_v = w2[:].rearrange("(a p) d -> p a d", p=P)
    yT_vs = [t[:].rearrange("(a p) c -> p a c", p=P) for t in yTs]

    with tile.TileContext(nc) as tc:
        with ExitStack() as ctx:
            wpool = ctx.enter_context(tc.tile_pool(name="weights", bufs=1))
            xpool = ctx.enter_context(tc.tile_pool(name="xin", bufs=1))
            hpool = ctx.enter_context(tc.tile_pool(name="hid", bufs=2))
            ypool = ctx.enter_context(tc.tile_pool(name="yout", bufs=1))
            ph_pool = ctx.enter_context(
                tc.tile_pool(name="psum_h", bufs=2, space="PSUM")
            )
            py_pool = ctx.enter_context(
                tc.tile_pool(name="psum_y", bufs=4, space="PSUM")
            )
            pp_pool = ctx.enter_context(
                tc.tile_pool(name="psum_probe", bufs=1, space="PSUM")
            )

            # The matmul codegen allows only ONE sync-wait command. A "probe"
            # is a tiny PE matmul issued right after an input DMA: it carries
            # that DMA-queue wait on PE's vector clock, so the real matmuls
            # reading the data need no DMA wait of their own.
            probe_ps = pp_pool.tile([P, 8], F32, tag="pp", name="probe_ps")
            probe_n = [0]

            def probe(ap2d):
                nc.tensor.matmul(
                    probe_ps[:1, :8],
                    ap2d[:, :1],
                    ap2d[:, :8],
                    start=True,
                    stop=True,
                )
                probe_n[0] += 1

            # Resident weights: 8 tiles of [128, 1024] each for w1 and w2.
            # x^T is fully resident too (no slot reuse -> minimal sync waits);
            # its DMA is split per token-tile so early matmuls start sooner.
            w1_sb = []
            w2_sb = []
            for c in range(DC):
                t1 = wpool.tile([P, H], MM_DT, tag=f"w1_{c}", name=f"w1_{c}")
                nc.sync.dma_start(out=t1, in_=w1_v[:, c, :])
                probe(t1)
                w1_sb.append(t1)
            x_sb = xpool.tile([P, DC, C], MM_DT, tag="x", name="x_all")
            col = 0
            for tt, N in enumerate(tiles):
                nc.sync.dma_start(
                    out=x_sb[:, :, col : col + N], in_=xg_v[:, :, col : col + N]
                )
                probe(x_sb[:, 0, col : col + N])
                if tt == 0:
                    for c in range(HC):
                        t2 = wpool.tile([P, D], MM_DT, tag=f"w2_{c}", name=f"w2_{c}")
                        nc.sync.dma_start(out=t2, in_=w2_v[:, c, :])
                        probe(t2)
                        w2_sb.append(t2)
                col += N

            col = 0
            for tt, N in enumerate(tiles):
                csl = slice(col, col + N)
                col += N

                # First layer + relu^2 (in place) -> h_sb [128, 8(hc), N]
                h_sb = hpool.tile([P, HC, N], MM_DT, tag="h", name=f"h_{tt}")
                for hc in range(HC):
                    ph = ph_pool.tile([P, N], F32, tag="ph", name=f"ph_{tt}_{hc}")
                    for dc in range(DC):
                        nc.tensor.matmul(
                            ph,
                            w1_sb[dc][:, bass.ts(hc, P)],
                            x_sb[:, dc, csl],
                            start=(dc == 0),
                            stop=(dc == DC - 1),
                        )
                    hs = h_sb[:, hc, :]
                    nc.vector.tensor_scalar_max(hs, ph, 0.0)
                    nc.vector.tensor_mul(hs, hs, hs)

                # Second layer: one d-chunk (one PSUM bank) per group;
                # copies gather into one SBUF tile, one DMA per token-tile
                # into its own output tensor (no cross-queue WAW waits).
                y_sb = ypool.tile([P, DC, N], F32, tag="y", name=f"y_{tt}")
                # DVE pre-op: absorbs the WAR wait on y_sb (previous tile's
                # outbound DMA) so each real copy carries only its PE wait.
                nc.vector.tensor_copy(y_sb[:1, 0, :1], y_sb[:1, 0, :1])
                for dc in range(DC):
                    py = py_pool.tile([P, N], F32, tag="py", name=f"py_{tt}_{dc}")
                    for hc in range(HC):
                        nc.tensor.matmul(
                            py,
                            w2_sb[hc][:, bass.ts(dc, P)],
                            h_sb[:, hc, :],
                            start=(hc == 0),
                            stop=(hc == HC - 1),
                        )
                    nc.vector.tensor_copy(y_sb[:, dc, :], py)
                nc.gpsimd.dma_start(out=yT_vs[tt], in_=y_sb)

    _split_multi_waits(nc)
    return nc


def _split_multi_waits(nc):
    """This walrus build allows only ONE sync-wait command per instruction.
    Hoist all but the last wait of any multi-wait instruction onto fresh
    same-engine NoOps inserted immediately before it (sequencer executes
    them in order, so semantics are identical)."""
    n = [0]
    for f in nc.m.functions:
        for blk in f.blocks:
            out = []
            for inst in blk.instructions:
                si = inst.sync_info
                if si is not None and len(si.on_wait) > 1:
                    waits = list(si.on_wait)
                    for w in waits[:-1]:
                        nop = mybir.InstNoOp(
                            name=f"I-waitsplit-{n[0]}", ins=[], outs=[]
                        )
                        n[0] += 1
                        nop.engine = inst.engine
                        nop.sync_info = mybir.SyncInfo(
                            on_wait=[w], on_update=[]
                        )
                        out.append(nop)
                    inst.sync_info = mybir.SyncInfo(
                        on_wait=[waits[-1]], on_update=list(si.on_update)
                    )
                out.append(inst)
            blk.instructions = out


def _route(x_flat, router_w, expert_bias):
    """Replicates the reference router in numpy float32."""
    logits = x_flat @ router_w.T  # [T, E]
    probs = 1.0 / (1.0 + np.exp(-logits, dtype=np.float32))
    scores = probs + expert_bias[None, :]
    # jax.lax.top_k: descending order, ties -> lower index first
    sel = np.argsort(-scores, axis=-1, kind="stable")[:, :K]  # [T, K]
    topw = np.take_along_axis(probs, sel, axis=-1)
    topw = topw / (topw.sum(-1, keepdims=True) + 1e-20)
    return logits, probs, sel, topw


def kernel(x, router_w, w1, w2, expert_bias):
    x = np.asarray(x, dtype=np.float32)
    router_w = np.asarray(router_w, dtype=np.float32)
    w1 = np.asarray(w1, dtype=np.float32)
    w2 = np.asarray(w2, dtype=np.float32)
    expert_bias = np.asarray(expert_bias, dtype=np.float32)

    b, s, d = x.shape
    x_flat = x.reshape(-1, d)
    n_tok = x_flat.shape[0]

    logits, probs, sel, topw = _route(x_flat, router_w, expert_bias)

    # Token dispatch: gather per-expert token lists
    idx = [np.nonzero((sel == e).any(axis=-1))[0] for e in range(E)]
    counts = [len(i) for i in idx]
    max_count = max(max(counts), 1)

    # Capacity: round up to a multiple of 256; tile widths 512 with an
    # optional trailing 256 (N>=256 keeps float32r at full PE rate).
    C = -(-max_count // 256) * 256
    if C % 512 == 256 and C > 256:
        tiles = [512] * (C // 512) + [256]
    elif C == 256:
        C = 512
        tiles = [512]
    else:
        tiles = [512] * (C // 512)

    key = (C, tuple(tiles))
    if key not in _program_cache:
        _program_cache[key] = _build_program(C, tiles)
    nc = _program_cache[key]

    xT = np.ascontiguousarray(x_flat.T)  # [D, T]
    in_maps = []
    for e in range(E):
        xg = np.zeros((D, C), dtype=np.float32)
        xg[:, : counts[e]] = xT[:, idx[e]]
        in_maps.append(
            {
                "xg": xg,
                "w1": np.ascontiguousarray(w1[:, e * H : (e + 1) * H]),
                "w2": np.ascontiguousarray(w2[e * H : (e + 1) * H, :]),
            }
        )

    res = run_bass_kernel_spmd(nc, in_maps, list(range(NCORES)))
    global LAST_RESULTS
    LAST_RESULTS = res

    # Unshard: scatter-add with combine weights
    out_flat = np.zeros((n_tok, d), dtype=np.float32)
    for e in range(E):
        if counts[e] == 0:
            continue
        yT_e = np.concatenate(
            [res.results[e][f"yT_{tt}"] for tt in range(len(tiles))], axis=1
        )  # [D, C]
        c_e = np.where(sel[idx[e]] == e, topw[idx[e]], 0.0).sum(-1)
        out_flat[idx[e]] += c_e[:, None].astype(np.float32) * yT_e[:, : counts[e]].T

    output = out_flat.reshape(b, s, d)

    # Aux losses (host, matching the reference formulas)
    m = logits.max(-1)
    lse = m + np.log(np.exp(logits - m[:, None]).sum(-1))
    router_z_loss = np.float32(np.mean(np.square(lse)))

    sel_flat = sel.reshape(-1)
    tokens_per_expert = np.zeros(E, np.float32)
    np.add.at(tokens_per_expert, sel_flat, 1.0)
    f_i = (tokens_per_expert / tokens_per_expert.sum()).astype(np.float32)

    sel_seq = sel.reshape(b, s * K)
    f_seq = np.zeros((b, E), np.float32)
    for i in range(b):
        np.add.at(f_seq[i], sel_seq[i], 1.0)
    f_seq /= s * K
    p_seq = probs.reshape(b, s, E).mean(axis=1)
    load_balance_loss = np.float32(E * (f_seq * p_seq).sum(-1).mean())

    compute_loss = np.float32(probs.sum(-1).mean())

    return output, router_z_loss, load_balance_loss, compute_loss, f_i
